# revision 4
# baseline (speedup 1.0000x reference)
"""AttnBlock (GroupNorm + single-head 1x1-conv attention + residual) on 8 TRN2 cores.

Sharding: core c handles batch b = c//2, query-token half c%2 (2048 of 4096
tokens). Each core computes GN + attention for its query half against all keys
of its batch element, returning [512, 2048]; host reassembles [4,512,64,64].

All heavy matmuls run in fp8e4m3 with DoubleRow perf mode (2 contraction rows
per cycle), halving PE streaming time vs f32r. Scales are arranged so every
fp8 tensor sits in e4m3's normal range:
  x8 = fp8(x)                      (std 1)
  mt8 = fp8(256*scale*wq^T wk * a) (std ~0.5)   a = per-channel GN scale
  wv8 = fp8(16*wv^T * a)           (std ~0.7)
  wp8 = fp8(16*wp^T)               (std ~0.7)
  q8  = fp8(a * (mt8-matmul + consts))  (std ~11)
  p8  = fp8(exp(S/256 - 3.0))      (<= ~70; softmax shift cancels)
  vT8 = fp8(V-matmul psum)         (std ~16)
  on8 = fp8(4*O/L)                 (std ~1.7)
GroupNorm is folded into the weights on device: the S/V matmuls consume raw
x8; the GN affine (a,b) enters via weight row-scaling plus small matvec
constants (b-terms along the softmax axis cancel exactly; the rest folds into
the output bias). exp bias of -3.0 and the 1/L normalization are exact
softmax-invariant shifts. The j-loop is software-pipelined (O-matmuls trail
S^T by DEPTH pairs) and ACT runs only Exp inside it (no act-table thrash).
"""

import numpy as np

B, C, HW = 4, 512, 64
N = HW * HW            # 4096 tokens
NQ = N // 2            # 2048 query tokens per core
NT = C // 128          # 4 channel tiles
NJ = N // 128          # 32 key chunks
NPAIR = NJ // 2        # 16 key-chunk pairs (DoubleRow)
NBLK = NQ // 512       # 4 query blocks of 512
NG = 32                # groups
EPS = 1e-6
SCALE = 1.0 / np.sqrt(C)

_CACHE = {}


def _build_nc(reps=1):
    import contextlib
    import concourse.bass as bass
    import concourse.mybir as mybir
    import concourse.tile as tile
    import concourse.bacc as bacc

    f32 = mybir.dt.float32
    f32r = mybir.dt.float32r
    fp8 = mybir.dt.float8e4
    AF = mybir.ActivationFunctionType
    OP = mybir.AluOpType
    DR = mybir.MatmulPerfMode.DoubleRow

    nc = bacc.Bacc("TRN2", target_bir_lowering=False, debug=False, num_devices=8)

    x_d = nc.dram_tensor("x", [C, N], f32, kind="ExternalInput").ap()
    mtf_d = nc.dram_tensor("mtf", [C, C], f32, kind="ExternalInput").ap()
    wvtf_d = nc.dram_tensor("wvtf", [C, C], f32, kind="ExternalInput").ap()
    wptf_d = nc.dram_tensor("wptf", [C, C], f32, kind="ExternalInput").ap()
    mvec_d = nc.dram_tensor("mvec", [128, NT], f32, kind="ExternalInput").ap()
    bpp_d = nc.dram_tensor("bpp", [128, NT], f32, kind="ExternalInput").ap()
    gnsc_d = nc.dram_tensor("gnsc", [128, NT], f32, kind="ExternalInput").ap()
    gnbi_d = nc.dram_tensor("gnbi", [128, NT], f32, kind="ExternalInput").ap()
    ind_d = nc.dram_tensor("ind", [C, NG], f32, kind="ExternalInput").ap()
    emat_d = nc.dram_tensor("emat", [NG, 128], f32, kind="ExternalInput").ap()
    tmask_d = nc.dram_tensor("tmask", [NG, NT], f32, kind="ExternalInput").ap()
    y_d = nc.dram_tensor("y", [C, NQ], f32, kind="ExternalOutput").ap()

    x_t = x_d.rearrange("(t p) n -> t p n", p=128)
    y_t = y_d.rearrange("(t p) n -> t p n", p=128)

    with tile.TileContext(nc) as tc:
        with (
            tc.tile_pool(name="xpool", bufs=1) as xpool,
            tc.tile_pool(name="x8pool", bufs=1) as x8pool,
            tc.tile_pool(name="vpool", bufs=1) as vpool,
            tc.tile_pool(name="wpool", bufs=1) as wpool,
            tc.tile_pool(name="w8pool", bufs=1) as w8pool,
            tc.tile_pool(name="cpool", bufs=1) as cpool,
            tc.tile_pool(name="gn", bufs=1) as gn,
            tc.tile_pool(name="qt", bufs=2) as qtp,
            tc.tile_pool(name="pt", bufs=8) as ptp,
            tc.tile_pool(name="fin", bufs=1) as finp,
            tc.tile_pool(name="psA", bufs=1, space="PSUM") as psA,
            tc.tile_pool(name="psB", bufs=1, space="PSUM") as psB,
            tc.tile_pool(name="psL", bufs=1, space="PSUM") as psL,
            tc.tile_pool(name="psO", bufs=1, space="PSUM") as psO,
        ):
            with (tc.For_i(0, reps, 1) if reps > 1 else contextlib.nullcontext()):
                # ---- load x + quantize to fp8 + per-chunk stats ----
                x_sb = xpool.tile([128, NT, N], f32, name="x_sb", tag="x_sb")
                x8 = x8pool.tile([128, NT, N], fp8, name="x8", tag="x8")
                for t in range(NT):
                    for s in range(8):
                        sl = slice(s * 512, (s + 1) * 512)
                        nc.sync.dma_start(x_sb[:, t, sl], x_t[t][:, sl])

                # ---- load weights/constants ----
                mtf_sb = wpool.tile([128, NT, C], f32)
                wvtf_sb = wpool.tile([128, NT, C], f32)
                wptf_sb = wpool.tile([128, NT, C], f32)
                nc.sync.dma_start(mtf_sb[:], mtf_d.rearrange("(t p) n -> p t n", p=128))
                nc.sync.dma_start(wvtf_sb[:], wvtf_d.rearrange("(t p) n -> p t n", p=128))
                nc.sync.dma_start(wptf_sb[:], wptf_d.rearrange("(t p) n -> p t n", p=128))
                mvec_sb = cpool.tile([128, NT], f32)
                bpp_sb = cpool.tile([128, NT], f32)
                gnsc_sb = cpool.tile([128, NT], f32)
                gnbi_sb = cpool.tile([128, NT], f32)
                ind_sb = cpool.tile([128, NT, NG], f32)
                emat_sb = cpool.tile([NG, 128], f32)
                tmask_sb = cpool.tile([NG, NT], f32)
                nc.sync.dma_start(mvec_sb[:], mvec_d[:])
                nc.sync.dma_start(bpp_sb[:], bpp_d[:])
                nc.sync.dma_start(gnsc_sb[:], gnsc_d[:])
                nc.sync.dma_start(gnbi_sb[:], gnbi_d[:])
                nc.sync.dma_start(ind_sb[:], ind_d.rearrange("(t p) g -> p t g", p=128))
                nc.sync.dma_start(emat_sb[:], emat_d[:])
                nc.sync.dma_start(tmask_sb[:], tmask_d[:])

                # ---- GroupNorm stats + x8 quantize (overlapped with DMA) ----
                stats3 = []
                for t in range(NT):
                    bnb = gn.tile([128, 8, 6], f32, name=f"bnb{t}", tag="bnb", bufs=2)
                    for s in range(8):
                        sl = slice(s * 512, (s + 1) * 512)
                        nc.vector.bn_stats(bnb[:, s, :], x_sb[:, t, sl])
                        if (t + s) % 2 == 0:
                            nc.scalar.copy(x8[:, t, sl], x_sb[:, t, sl])
                        else:
                            nc.vector.tensor_copy(x8[:, t, sl], x_sb[:, t, sl])
                    mv = gn.tile([128, 2], f32, name=f"mv{t}", tag="mv", bufs=2)
                    nc.vector.bn_aggr(mv[:], bnb[:])
                    s3 = gn.tile([128, 3], f32, name=f"s3_{t}", tag=f"s3_{t}")
                    nc.vector.tensor_copy(s3[:, 0:2], mv[:])
                    nc.scalar.square(s3[:, 2:3], mv[:, 0:1])
                    stats3.append(s3)
                # group aggregation: [32, 3] = sum_c ind[c, g] * [mean, var, mean^2]
                ps_g = psO.tile([NG, 3], f32, tag="psO0")
                for t in range(NT):
                    nc.tensor.matmul(ps_g[:], ind_sb[:, t, :], stats3[t][:],
                                     start=(t == 0), stop=(t == NT - 1))
                sg = gn.tile([NG, 3], f32)
                nc.vector.tensor_copy(sg[:], ps_g[:])
                msq = gn.tile([NG, 1], f32)
                nc.scalar.square(msq[:], sg[:, 0:1])
                vg = gn.tile([NG, 1], f32)
                nc.vector.tensor_add(vg[:], sg[:, 1:2], sg[:, 2:3])
                nc.vector.tensor_sub(vg[:], vg[:], msq[:])
                eps_t = gn.tile([NG, 1], f32)
                nc.vector.memset(eps_t[:], EPS)
                std = gn.tile([NG, 1], f32)
                nc.scalar.activation(std[:], vg[:], AF.Sqrt, bias=eps_t[:])
                inv = gn.tile([NG, 1], f32)
                nc.vector.reciprocal(inv[:], std[:])
                mcol = sg[:, 0:1]
                rmat = gn.tile([NG, 2 * NT], f32)
                nc.vector.tensor_scalar_mul(rmat[:, 0:NT], tmask_sb[:], inv[:])
                nc.vector.tensor_scalar_mul(rmat[:, NT:2 * NT], tmask_sb[:], mcol[:])
                ps_e = psO.tile([128, 2 * NT], f32, tag="psO1")
                nc.tensor.matmul(ps_e[:], emat_sb[:], rmat[:], start=True, stop=True)
                a_pc = gn.tile([128, NT], f32)
                b_pc = gn.tile([128, NT], f32)
                nc.vector.tensor_mul(a_pc[:], gnsc_sb[:], ps_e[:, 0:NT])
                nc.vector.tensor_mul(b_pc[:], ps_e[:, NT:2 * NT], a_pc[:])
                nc.vector.tensor_sub(b_pc[:], gnbi_sb[:], b_pc[:])

                # ---- fold GN scale into fp8 weights ----
                mt8 = w8pool.tile([128, NT, C], fp8)
                wv8 = w8pool.tile([128, NT, C], fp8)
                wp8 = w8pool.tile([128, NT, C], fp8)
                for t in range(NT):
                    nc.vector.tensor_scalar_mul(mt8[:, t, :], mtf_sb[:, t, :],
                                                a_pc[:, t:t + 1])
                    nc.vector.tensor_scalar_mul(wv8[:, t, :], wvtf_sb[:, t, :],
                                                a_pc[:, t:t + 1])
                    nc.scalar.copy(wp8[:, t, :], wptf_sb[:, t, :])

                # ---- GN-bias matvec constants (tiny f32r matmuls) ----
                # mb[o]  = sum_c mtf[c,o]*b_c     (= 256*scale*(wq^T wk)^T b)
                # cvb[o] = sum_c wvtf[c,o]*b_c    (= 16*(wv b))
                # bppX   = bpp + (wptf^T cvb)/256 (= bpp + wp wv b)
                mb = gn.tile([128, NT], f32)
                cvb = gn.tile([128, NT], f32)
                bppX = gn.tile([128, NT], f32)
                for co in range(NT):
                    csel = slice(co * 128, (co + 1) * 128)
                    ps_mb = psB.tile([128, 2], f32, tag="psB")
                    for t in range(NT):
                        nc.tensor.matmul(ps_mb[:, 0:1],
                                         mtf_sb[:, t, csel],
                                         b_pc[:, t:t + 1],
                                         start=(t == 0), stop=(t == NT - 1))
                    for t in range(NT):
                        nc.tensor.matmul(ps_mb[:, 1:2],
                                         wvtf_sb[:, t, csel],
                                         b_pc[:, t:t + 1],
                                         start=(t == 0), stop=(t == NT - 1))
                    nc.vector.tensor_copy(mb[:, co:co + 1], ps_mb[:, 0:1])
                    nc.vector.tensor_copy(cvb[:, co:co + 1], ps_mb[:, 1:2])
                for co in range(NT):
                    csel = slice(co * 128, (co + 1) * 128)
                    ps_pb = psB.tile([128, 2], f32, tag="psB")
                    for t in range(NT):
                        nc.tensor.matmul(ps_pb[:, 0:1],
                                         wptf_sb[:, t, csel],
                                         cvb[:, t:t + 1],
                                         start=(t == 0), stop=(t == NT - 1))
                    nc.vector.tensor_scalar(bppX[:, co:co + 1], ps_pb[:, 0:1],
                                            1.0 / 256.0, bpp_sb[:, co:co + 1],
                                            OP.mult, OP.add)
                # amv[o] = a_o*(mb[o] + mvec[o])  (q-side additive const)
                amv = gn.tile([128, NT], f32)
                nc.vector.tensor_add(amv[:], mb[:], mvec_sb[:])
                nc.vector.tensor_mul(amv[:], amv[:], a_pc[:])
                ebias = gn.tile([128, 1], f32)
                nc.vector.memset(ebias[:], -3.0)
                ones8 = gn.tile([128, 2, 16], fp8)
                nc.vector.memset(ones8[:], 1.0)

                # ---- V^T: 32 chunks [128 keys, 512 c] in one fp8 tile ----
                vT8 = vpool.tile([128, NJ, 512], fp8, name="vT8", tag="vT8")
                ps_v = psA.tile([128, 2, 512], f32, tag="psA")
                for js in range(NJ):
                    jsel = slice(js * 128, (js + 1) * 128)
                    for u in range(2):
                        nc.tensor.matmul(ps_v[:, js % 2, :],
                                         x8[:, 2 * u:2 * u + 2, jsel],
                                         wv8[:, 2 * u:2 * u + 2, :],
                                         start=(u == 0), stop=(u == 1), perf_mode=DR)
                    if js % 2 == 0:
                        nc.scalar.copy(vT8[:, js, :], ps_v[:, js % 2, :])
                    else:
                        nc.vector.tensor_copy(vT8[:, js, :], ps_v[:, js % 2, :])

                # ---- attention over 4 query blocks of 512 ----
                def emit_q8(ib):
                    isel = slice(ib * 512, (ib + 1) * 512)
                    q8 = qtp.tile([128, NT, 512], fp8, name=f"q8_{ib}", tag="q8")
                    for co in range(NT):
                        csel = slice(co * 128, (co + 1) * 128)
                        ps_q = psB.tile([128, 512], f32, tag="psB")
                        for u in range(2):
                            nc.tensor.matmul(ps_q[:],
                                             mt8[:, 2 * u:2 * u + 2, csel],
                                             x8[:, 2 * u:2 * u + 2, isel],
                                             start=(u == 0), stop=(u == 1),
                                             perf_mode=DR)
                        nc.vector.tensor_scalar(q8[:, co, :], ps_q[:],
                                                a_pc[:, co:co + 1],
                                                amv[:, co:co + 1],
                                                OP.mult, OP.add)
                    return q8

                next_q8 = emit_q8(0)
                for ib in range(NBLK):
                    isel = slice(ib * 512, (ib + 1) * 512)
                    q8 = next_q8
                    ps_o = [psO.tile([128, 512], f32, name=f"o{ib}_{cs}",
                                     tag=f"psO{cs}") for cs in range(NT)]
                    ps_s = psA.tile([128, 2, 512], f32, tag="psA")
                    ps_l = psL.tile([1, 512], f32, tag="psL")
                    DEPTH = 3
                    pend = []

                    def emit_o(pr, p8):
                        for cs in range(NT):
                            nc.tensor.matmul(
                                ps_o[cs][:],
                                vT8[:, 2 * pr:2 * pr + 2, cs * 128:(cs + 1) * 128],
                                p8[:], start=(pr == 0), stop=(pr == NPAIR - 1),
                                perf_mode=DR)
                        nc.tensor.matmul(ps_l[:], ones8[:, :, 0:1], p8[:],
                                         start=(pr == 0), stop=(pr == NPAIR - 1),
                                         perf_mode=DR)

                    for pr in range(NPAIR):
                        for s in range(2):
                            jsel = slice((2 * pr + s) * 128, (2 * pr + s + 1) * 128)
                            for u in range(2):
                                nc.tensor.matmul(ps_s[:, s, :],
                                                 x8[:, 2 * u:2 * u + 2, jsel],
                                                 q8[:, 2 * u:2 * u + 2, :],
                                                 start=(u == 0), stop=(u == 1),
                                                 perf_mode=DR)
                        p8 = ptp.tile([128, 2, 512], fp8, tag="pt")
                        nc.scalar.activation(p8[:], ps_s[:], AF.Exp,
                                             bias=ebias[:], scale=1.0 / 256.0)
                        pend.append((pr, p8))
                        if len(pend) > DEPTH:
                            emit_o(*pend.pop(0))
                    for item in pend:
                        emit_o(*item)
                    if ib + 1 < NBLK:
                        next_q8 = emit_q8(ib + 1)
                    lrec = finp.tile([1, 512], f32, tag="lrec", bufs=2)
                    nc.vector.reciprocal(lrec[:], ps_l[:])
                    lb = finp.tile([128, 512], f32, tag="lb", bufs=2)
                    nc.gpsimd.partition_broadcast(lb[:], lrec[:])
                    on8 = finp.tile([128, NT, 512], fp8, name="on8", tag="on8")
                    for cs in range(NT):
                        nc.vector.scalar_tensor_tensor(
                            on8[:, cs, :], ps_o[cs][:], 4.0, lb[:],
                            OP.mult, OP.mult)
                    # proj + bias + residual
                    out_sb = finp.tile([128, NT, 512], f32, tag="out", bufs=2)
                    for os_ in range(NT):
                        csel = slice(os_ * 128, (os_ + 1) * 128)
                        ps_p = psB.tile([128, 512], f32, tag="psB")
                        for u in range(2):
                            nc.tensor.matmul(ps_p[:],
                                             wp8[:, 2 * u:2 * u + 2, csel],
                                             on8[:, 2 * u:2 * u + 2, :],
                                             start=(u == 0), stop=(u == 1),
                                             perf_mode=DR)
                        nc.vector.tensor_scalar(
                            out_sb[:, os_, :], ps_p[:],
                            1.0 / 1024.0, bppX[:, os_:os_ + 1],
                            OP.mult, OP.add)
                        nc.vector.tensor_add(out_sb[:, os_, :], out_sb[:, os_, :],
                                             x_sb[:, os_, isel])
                    nc.sync.dma_start(y_t[:, :, isel].rearrange("t p n -> p t n"),
                                      out_sb[:])
    nc.compile()
    return nc


def _host_prep(gn_scale, gn_bias, wq, bq, wk, bk, wv, bv, wp, bp):
    f = np.float32

    def pc(v):  # [512] -> [128, 4] channel layout (c = t*128 + p)
        return np.ascontiguousarray(v.reshape(NT, 128).T).astype(f)

    wq64, wk64, wv64, wp64 = (np.asarray(w, np.float64) for w in (wq, wk, wv, wp))
    mtf = (256.0 * SCALE * (wq64.T @ wk64)).astype(f)             # [c_in, c_out]
    mvec = pc((256.0 * SCALE * (wk64.T @ np.asarray(bq, np.float64))).astype(f))
    bpp = pc((np.asarray(bp, np.float64) + wp64 @ np.asarray(bv, np.float64)).astype(f))
    wvtf = np.ascontiguousarray(16.0 * wv64.T).astype(f)
    wptf = np.ascontiguousarray(16.0 * wp64.T).astype(f)

    ind = np.zeros((C, NG), f)
    ind[np.arange(C), np.arange(C) // 16] = 1.0 / 16.0
    emat = np.zeros((NG, 128), f)
    for g in range(NG):
        for p in range(128):
            if p // 16 == g % 8:
                emat[g, p] = 1.0
    tmask = np.zeros((NG, NT), f)
    for g in range(NG):
        tmask[g, g // 8] = 1.0
    return dict(
        mtf=mtf, wvtf=wvtf, wptf=wptf, mvec=mvec, bpp=bpp,
        gnsc=pc(np.asarray(gn_scale, f)), gnbi=pc(np.asarray(gn_bias, f)),
        ind=ind, emat=emat, tmask=tmask,
    )


def kernel(hidden_states, gn_scale, gn_bias, wq, bq, wk, bk, wv, bv, wp, bp):
    from concourse.bass_utils import run_bass_kernel_spmd

    if "nc" not in _CACHE:
        _CACHE["nc"] = _build_nc()
    nc = _CACHE["nc"]

    shared = _host_prep(gn_scale, gn_bias, wq, bq, wk, bk, wv, bv, wp, bp)
    x = np.asarray(hidden_states, np.float32).reshape(B, C, N)

    in_maps = []
    for c in range(8):
        b, half = c // 2, c % 2
        xb = x[b]
        if half:
            # roll so this core's query tokens sit at [0, 2048)
            xb = np.concatenate([xb[:, NQ:], xb[:, :NQ]], axis=1)
        m = dict(shared)
        m["x"] = np.ascontiguousarray(xb)
        in_maps.append(m)

    res = run_bass_kernel_spmd(nc, in_maps, list(range(8)))

    out = np.empty((B, C, N), np.float32)
    for c in range(8):
        b, half = c // 2, c % 2
        out[b][:, half * NQ:(half + 1) * NQ] = res.results[c]["y"]
    return out.reshape(B, C, HW, HW)
